# revision 23
# baseline (speedup 1.0000x reference)
"""CRF loss kernel for Trainium2 (8 NeuronCores, SPMD data-parallel over batch).

Per core (local batch 64), V3 design:
  The log-partition forward algorithm runs in probability space, split into a
  forward chain (alpha, t=0..255) and a backward chain (beta, t=511..256)
  stitched exactly via Z = sum_j alpha_255[j] * beta_255[j].  The two chains
  are STACKED on the 128 SBUF partitions (fwd on 0..63, bwd on 64..127) and
  advanced by a single matmul against a constant block-diagonal weight
  W = [[exp(trans), 0], [0, exp(trans)^T]], followed by one DVE multiply with
  Q[t] = exp(emis^T - SHIFT) (top half in forward time order, bottom half
  time-reversed, prepared host-side).  The local batch is split into two
  32-wide pair-chains so the two chains hide each other's PE->DVE->PE
  latency.  Every K steps each chain renormalizes by a power of two from its
  row-0 exponent bits (DVE bitwise ops + tiny broadcast matmuls); scale logs
  are restored at the end.
  Numerator emission-sum: sum_t emis[b,t,tags[b,t]] via chunked DVE
  multiply+reduce of (emis * onehot) in a 128-partition packed natural
  layout, folded across partition halves with a small matmul.  The
  start/transition/end lookups (tiny tags/transitions tensors only) are
  added on the host.
"""

import os
import sys

import numpy as np
import ml_dtypes

for _p in ("/opt/trn_rl_repo", "/opt/pypackages"):
    if os.path.isdir(_p) and _p not in sys.path:
        sys.path.append(_p)

import concourse.bass as bass
import concourse.bacc as bacc
import concourse.mybir as mybir
import concourse.tile as tile
from concourse.alu_op_type import AluOpType
from contextlib import ExitStack

B, T, C = 512, 512, 64
NCORES = 8
BLOC = B // NCORES  # 64
SHIFT = 6.0
K_RENORM = 32
NCHAIN = 2            # pair-chains (batch split within a core)
TCH = 64              # slot chunk for Qpair DMA / exp
NUM_TCH = 16          # t-half chunk per numerator DVE op
NUM_DMA_TCH = 64      # t-half chunk per numerator DMA

AF = mybir.ActivationFunctionType
bf16 = ml_dtypes.bfloat16


def build_crf_program(T=T, K=K_RENORM):
    dt = mybir.dt
    f32, b16, u16 = dt.float32, dt.bfloat16, dt.uint16
    assert T % 2 == 0
    H = T // 2          # slots; fwd covers t=0..H-1, bwd t=T-1..H
    BG = BLOC // NCHAIN  # 32
    RROWS = 16

    nc = bacc.Bacc("TRN2", target_bir_lowering=False, debug=False, num_devices=NCORES)
    # [128, H, BLOC]: top = emis^T t=0..H-1, bottom = emis^T t=T-1..H (reversed)
    emisP = nc.dram_tensor("emisP", [2 * C, H, BLOC], b16, kind="ExternalInput").ap()
    # numerator natural layout, partition p = th*BLOC + b, free (t', c)
    emis_nat = nc.dram_tensor("emis_nat", [2 * BLOC, H * C], b16, kind="ExternalInput").ap()
    oh_nat = nc.dram_tensor("oh_nat", [2 * BLOC, H * C], b16, kind="ExternalInput").ap()
    trans_d = nc.dram_tensor("trans", [C, C], f32, kind="ExternalInput").ap()
    transT_d = nc.dram_tensor("transT", [C, C], f32, kind="ExternalInput").ap()
    startend_d = nc.dram_tensor("startend", [2 * C, 1], f32, kind="ExternalInput").ap()
    ident_d = nc.dram_tensor("ident", [C, C], b16, kind="ExternalInput").ap()
    fold_d = nc.dram_tensor("foldmat", [2 * BLOC, BLOC], f32, kind="ExternalInput").ap()
    out_logZ = nc.dram_tensor("out_logZ", [1, BLOC], f32, kind="ExternalOutput").ap()
    out_esum = nc.dram_tensor("out_esum", [1, BLOC], f32, kind="ExternalOutput").ap()

    with ExitStack() as ctx:
        tc = ctx.enter_context(tile.TileContext(nc))
        const = ctx.enter_context(tc.tile_pool(name="const", bufs=1))
        qpool = ctx.enter_context(tc.tile_pool(name="q", bufs=1))
        chunks = ctx.enter_context(tc.tile_pool(name="chunks", bufs=3))
        natp = ctx.enter_context(tc.tile_pool(name="natp", bufs=2))
        state = ctx.enter_context(tc.tile_pool(name="state", bufs=3))
        misc = ctx.enter_context(tc.tile_pool(name="misc", bufs=2))
        ps_s = ctx.enter_context(tc.tile_pool(name="ps_s", bufs=2, space="PSUM"))
        ps_bc = ctx.enter_context(tc.tile_pool(name="ps_bc", bufs=2, space="PSUM"))
        ps_z = ctx.enter_context(tc.tile_pool(name="ps_z", bufs=1, space="PSUM"))

        # ---- constants ----
        trans_sb = const.tile([C, C], f32)
        nc.sync.dma_start(trans_sb[:], trans_d)
        transT_sb = const.tile([2 * C, C], f32)
        nc.sync.dma_start(transT_sb[C:2 * C, :], transT_d)
        W = const.tile([2 * C, 2 * C], b16)
        nc.vector.memset(W[:], 0.0)
        nc.scalar.activation(W[0:C, 0:C], trans_sb[:], AF.Exp)
        nc.scalar.activation(W[C:2 * C, C:2 * C], transT_sb[C:2 * C, :], AF.Exp)

        startend_sb = const.tile([2 * C, 1], f32)
        nc.sync.dma_start(startend_sb[:], startend_d)
        expSE = const.tile([2 * C, 1], f32)
        nc.scalar.activation(expSE[:], startend_sb[:], AF.Exp)

        ident_pair = const.tile([2 * C, C], b16)
        nc.sync.dma_start(ident_pair[C:2 * C, :], ident_d)
        fold_sb = const.tile([2 * BLOC, BLOC], f32)
        nc.sync.dma_start(fold_sb[:], fold_d)

        ones1 = const.tile([1, C], b16)
        nc.vector.memset(ones1[:], 1.0)
        ones64 = const.tile([C, 1], b16)
        nc.vector.memset(ones64[:], 1.0)
        neg_shift = const.tile([2 * C, 1], f32)
        nc.vector.memset(neg_shift[:], -SHIFT)
        scales = const.tile([1, RROWS * BLOC], b16)
        nc.vector.memset(scales[:], 1.0)

        # ---- Qpair: [128, H*BLOC] ----
        Qt = qpool.tile([2 * C, H * BLOC], b16)
        # stage boundaries: small first chunk so slot 1 starts early
        bounds = [0]
        pos = 0
        while pos < H:
            step = 8 if pos == 0 else min(TCH, H - pos)
            step = min(step, H - pos)
            pos += step
            bounds.append(pos)
        for ch in range(len(bounds) - 1):
            lo, hi = bounds[ch], bounds[ch + 1]
            et = chunks.tile([2 * C, (hi - lo) * BLOC], b16, tag="emis")
            nc.sync.dma_start(
                et[:].rearrange("p (t b) -> p t b", t=hi - lo),
                emisP[:, lo:hi, :],
            )
            nc.scalar.activation(
                Qt[:, lo * BLOC:hi * BLOC], et[:], AF.Exp,
                bias=neg_shift[:, :1],
            )

        def q_slice(k, c):
            lo = k * BLOC + c * BG
            return Qt[:, lo:lo + BG]

        # ---- numerator ----
        num_tch = min(NUM_TCH, H)
        num_dma_tch = min(NUM_DMA_TCH, H)
        n_numops = H // num_tch
        num_parts = const.tile([2 * BLOC, n_numops], f32)
        num_emitted = [0]
        _nat = {}

        def emit_num_op():
            i = num_emitted[0]
            if i >= n_numops:
                return
            num_emitted[0] += 1
            dch = (i * num_tch) // num_dma_tch
            if _nat.get("ch") != dch:
                en = natp.tile([2 * BLOC, num_dma_tch * C], b16, tag="en")
                nc.sync.dma_start(
                    en[:], emis_nat[:, dch * num_dma_tch * C:(dch + 1) * num_dma_tch * C])
                on = natp.tile([2 * BLOC, num_dma_tch * C], b16, tag="on")
                nc.sync.dma_start(
                    on[:], oh_nat[:, dch * num_dma_tch * C:(dch + 1) * num_dma_tch * C])
                _nat["ch"] = dch
                _nat["tiles"] = (en, on)
            en, on = _nat["tiles"]
            off = (i * num_tch - dch * num_dma_tch) * C
            scr = misc.tile([2 * BLOC, num_tch * C], b16, tag="numscr")
            nc.gpsimd.tensor_tensor(scr[:], en[:, off:off + num_tch * C],
                                    on[:, off:off + num_tch * C], op=AluOpType.mult)
            scr2 = misc.tile([2 * BLOC, num_tch * C], b16, tag="numscr2")
            nc.scalar.activation(scr2[:], scr[:], AF.Copy,
                                 accum_out=num_parts[:, i:i + 1])

        # ---- init pair-chains (slot 0) ----
        p_cur = []
        for c in range(NCHAIN):
            p0 = state.tile([2 * C, BG], b16, tag=f"p{c}")
            nc.vector.tensor_scalar(p0[:], q_slice(0, c), expSE[:, :1], None,
                                    op0=AluOpType.mult)
            p_cur.append(p0)

        def renorm_prep(x_sb, row, c):
            """Extract power-of-2 scales from pair tile x rows 0 / C and
            broadcast them across partitions (runs off the critical path)."""
            srow_f = scales[:1, (2 * row) * BLOC + c * BG:(2 * row) * BLOC + c * BG + BG]
            srow_b = scales[:1, (2 * row + 1) * BLOC + c * BG:(2 * row + 1) * BLOC + c * BG + BG]
            nc.vector.tensor_scalar(srow_f.bitcast(u16), x_sb[:1, :].bitcast(u16),
                                    0x7F80, 0x7F80, op0=AluOpType.bitwise_and,
                                    op1=AluOpType.bitwise_xor)
            nc.vector.tensor_scalar(srow_b.bitcast(u16), x_sb[C:C + 1, :].bitcast(u16),
                                    0x7F80, 0x7F80, op0=AluOpType.bitwise_and,
                                    op1=AluOpType.bitwise_xor)
            bc = ps_bc.tile([2 * C, BG], f32, tag="bc")
            nc.tensor.matmul(bc[0:C, :], lhsT=ones1[:], rhs=srow_f,
                             start=True, stop=True)
            nc.tensor.matmul(bc[C:2 * C, :], lhsT=ones1[:], rhs=srow_b,
                             start=True, stop=True)
            return bc

        # ---- scan ----
        bc_pending = [None] * NCHAIN
        for k in range(1, H):
            for c in range(NCHAIN):
                s = ps_s.tile([2 * C, BG], f32, tag=f"s{c}")
                nc.tensor.matmul(s[:], lhsT=W[:], rhs=p_cur[c][:],
                                 start=True, stop=True)
                p_new = state.tile([2 * C, BG], b16, tag=f"p{c}")
                nc.vector.tensor_tensor(p_new[:], s[:], q_slice(k, c),
                                        op=AluOpType.mult)
                if k % K == 0:
                    bc = renorm_prep(p_new, k // K - 1, c)
                    p2 = state.tile([2 * C, BG], b16, tag=f"p{c}")
                    nc.vector.tensor_tensor(p2[:], p_new[:], bc[:],
                                            op=AluOpType.mult)
                    p_new = p2
                p_cur[c] = p_new
            if k % (H // n_numops) == (H // n_numops) - 1:
                emit_num_op()
        while num_emitted[0] < n_numops:
            emit_num_op()

        # ---- stitch: Z = sum_j alpha[j] * (E @ v)[j] per chain ----
        logZrow = misc.tile([1, BLOC], f32, tag="logZ")
        scales_ln = misc.tile([1, RROWS * BLOC], f32, tag="sln")
        nc.scalar.activation(scales_ln[:], scales[:1, :], AF.Ln)
        ssum = misc.tile([1, BLOC], f32, tag="ssum")
        nc.vector.tensor_reduce(
            ssum[:], scales_ln[:1, :].rearrange("p (r b) -> p b r", r=RROWS),
            mybir.AxisListType.X, AluOpType.add)
        for c in range(NCHAIN):
            s = ps_s.tile([2 * C, BG], f32, tag=f"s{c}")
            nc.tensor.matmul(s[:], lhsT=W[:], rhs=p_cur[c][:], start=True, stop=True)
            beta_hi = misc.tile([2 * C, BG], b16, tag="betahi")
            nc.vector.tensor_copy(beta_hi[C:2 * C, :], s[C:2 * C, :])
            blo = ps_bc.tile([C, BG], f32, tag="bc")
            nc.tensor.matmul(blo[:], lhsT=ident_pair[C:2 * C, :],
                             rhs=beta_hi[C:2 * C, :], start=True, stop=True)
            w = misc.tile([C, BG], b16, tag="w")
            nc.vector.tensor_tensor(w[:], blo[:], p_cur[c][0:C, :],
                                    op=AluOpType.mult)
            z = ps_z.tile([1, BG], f32, tag="z")
            nc.tensor.matmul(z[:], lhsT=ones64[:], rhs=w[:], start=True, stop=True)
            lnz = misc.tile([1, BG], f32, tag="lnz")
            nc.scalar.activation(lnz[:], z[:], AF.Ln)
            nc.vector.scalar_tensor_tensor(
                logZrow[:1, c * BG:(c + 1) * BG], lnz[:], float(SHIFT * T),
                ssum[:1, c * BG:(c + 1) * BG],
                op0=AluOpType.add, op1=AluOpType.subtract)
        nc.sync.dma_start(out_logZ, logZrow[:])

        # ---- numerator fold ----
        parts_red = misc.tile([2 * BLOC, 1], f32, tag="partsred")
        nc.vector.tensor_reduce(parts_red[:], num_parts[:], mybir.AxisListType.X,
                                AluOpType.add)
        ez = ps_z.tile([1, BLOC], f32, tag="z")
        nc.tensor.matmul(ez[:], lhsT=parts_red[:], rhs=fold_sb[:],
                         start=True, stop=True)
        esum_sb = misc.tile([1, BLOC], f32, tag="esum")
        nc.vector.tensor_copy(esum_sb[:], ez[:])
        nc.sync.dma_start(out_esum, esum_sb[:])

    nc.compile()
    return nc


_PROG_CACHE = {}


def _get_program(T_=T):
    if T_ not in _PROG_CACHE:
        _PROG_CACHE[T_] = build_crf_program(T=T_)
    return _PROG_CACHE[T_]


def host_prepare(emissions, tags, transitions, start_transitions, end_transitions,
                 T_=T):
    """Per-core input maps + host (tiny-tensor) numerator part."""
    H = T_ // 2
    in_maps = []
    trans_f = np.ascontiguousarray(transitions, dtype=np.float32)
    transT_f = np.ascontiguousarray(transitions.T, dtype=np.float32)
    startend = np.concatenate([start_transitions, end_transitions]).astype(
        np.float32).reshape(2 * C, 1)
    ident = np.eye(C, dtype=bf16)
    fold = np.tile(np.eye(BLOC, dtype=np.float32), (2, 1))
    cidx = np.arange(C, dtype=np.int32)
    tiny = np.zeros(B, np.float64)
    for c in range(NCORES):
        b0 = c * BLOC
        em = emissions[b0:b0 + BLOC, :T_, :]            # [Bl,T,C]
        emT = em.transpose(2, 1, 0)                     # [C,T,Bl]
        # top: t=0..H-1 ; bottom: t=T-1..H (time-reversed)
        emisP = np.concatenate([emT[:, :H, :], emT[:, ::-1, :][:, :H, :]], axis=0)
        emisP = np.ascontiguousarray(emisP).astype(bf16)
        emis_nat = np.ascontiguousarray(
            em.reshape(BLOC, 2, H * C).transpose(1, 0, 2).reshape(2 * BLOC, H * C)
        ).astype(bf16)
        tg = tags[b0:b0 + BLOC, :T_]                    # [Bl,T]
        oh = (tg[:, :, None] == cidx[None, None, :])    # [Bl,T,C]
        oh_nat = np.ascontiguousarray(
            oh.reshape(BLOC, 2, H * C).transpose(1, 0, 2).reshape(2 * BLOC, H * C)
        ).astype(bf16)
        in_maps.append({
            "emisP": emisP, "emis_nat": emis_nat, "oh_nat": oh_nat,
            "trans": trans_f, "transT": transT_f, "startend": startend,
            "ident": ident, "foldmat": fold,
        })
        tiny[b0:b0 + BLOC] = (
            start_transitions[tg[:, 0]].astype(np.float64)
            + np.take_along_axis(
                transitions[tg[:, :-1]], tg[:, 1:, None], axis=2)[:, :, 0].sum(1)
            + end_transitions[tg[:, -1]]
        )
    return in_maps, tiny


def kernel(emissions, tags, mask, transitions, start_transitions,
           end_transitions):
    from concourse.bass_utils import run_bass_kernel_spmd
    nc = _get_program()
    in_maps, tiny = host_prepare(emissions, tags, transitions,
                                 start_transitions, end_transitions)
    res = run_bass_kernel_spmd(nc, in_maps, core_ids=list(range(NCORES)))
    vals = np.zeros(B, np.float64)
    for c in range(NCORES):
        b0 = c * BLOC
        logZ = res.results[c]["out_logZ"].reshape(BLOC).astype(np.float64)
        esum = res.results[c]["out_esum"].reshape(BLOC).astype(np.float64)
        vals[b0:b0 + BLOC] = logZ - esum - tiny[b0:b0 + BLOC]
    return np.float32(np.mean(vals))


# revision 24
# speedup vs baseline: 1.0434x; 1.0434x over previous
"""CRF loss kernel for Trainium2 (8 NeuronCores, SPMD data-parallel over batch).

Per core (local batch 64), V3 design:
  The log-partition forward algorithm runs in probability space, split into a
  forward chain (alpha, t=0..255) and a backward chain (beta, t=511..256)
  stitched exactly via Z = sum_j alpha_255[j] * beta_255[j].  The two chains
  are STACKED on the 128 SBUF partitions (fwd on 0..63, bwd on 64..127) and
  advanced by a single matmul against a constant block-diagonal weight
  W = [[exp(trans), 0], [0, exp(trans)^T]], followed by one DVE multiply with
  Q[t] = exp(emis^T - SHIFT) (top half in forward time order, bottom half
  time-reversed, prepared host-side).  The local batch is split into two
  32-wide pair-chains so the two chains hide each other's PE->DVE->PE
  latency.  Every K steps each chain renormalizes by a power of two from its
  row-0 exponent bits (DVE bitwise ops + tiny broadcast matmuls); scale logs
  are restored at the end.
  Numerator emission-sum: sum_t emis[b,t,tags[b,t]] via chunked DVE
  multiply+reduce of (emis * onehot) in a 128-partition packed natural
  layout, folded across partition halves with a small matmul.  The
  start/transition/end lookups (tiny tags/transitions tensors only) are
  added on the host.
"""

import os
import sys

import numpy as np
import ml_dtypes

for _p in ("/opt/trn_rl_repo", "/opt/pypackages"):
    if os.path.isdir(_p) and _p not in sys.path:
        sys.path.append(_p)

import concourse.bass as bass
import concourse.bacc as bacc
import concourse.mybir as mybir
import concourse.tile as tile
from concourse.alu_op_type import AluOpType
from contextlib import ExitStack

B, T, C = 512, 512, 64
NCORES = 8
BLOC = B // NCORES  # 64
SHIFT = 6.0
K_RENORM = 32
NCHAIN = 2            # pair-chains (batch split within a core)
TCH = 64              # slot chunk for Qpair DMA / exp
NUM_TCH = 16          # t-half chunk per numerator DVE op
NUM_DMA_TCH = 64      # t-half chunk per numerator DMA

AF = mybir.ActivationFunctionType
bf16 = ml_dtypes.bfloat16


def build_crf_program(T=T, K=K_RENORM):
    dt = mybir.dt
    f32, b16, u16 = dt.float32, dt.bfloat16, dt.uint16
    assert T % 2 == 0
    H = T // 2          # slots; fwd covers t=0..H-1, bwd t=T-1..H
    BG = BLOC // NCHAIN  # 32
    RROWS = 16

    nc = bacc.Bacc("TRN2", target_bir_lowering=False, debug=False, num_devices=NCORES)
    # [128, H, BLOC]: top = emis^T t=0..H-1, bottom = emis^T t=T-1..H (reversed)
    emisP = nc.dram_tensor("emisP", [2 * C, H, BLOC], b16, kind="ExternalInput").ap()
    # numerator natural layout, partition p = th*BLOC + b, free (t', c)
    emis_nat = nc.dram_tensor("emis_nat", [2 * BLOC, H * C], b16, kind="ExternalInput").ap()
    oh_nat = nc.dram_tensor("oh_nat", [2 * BLOC, H * C], b16, kind="ExternalInput").ap()
    trans_d = nc.dram_tensor("trans", [C, C], f32, kind="ExternalInput").ap()
    transT_d = nc.dram_tensor("transT", [C, C], f32, kind="ExternalInput").ap()
    startend_d = nc.dram_tensor("startend", [2 * C, 1], f32, kind="ExternalInput").ap()
    ident_d = nc.dram_tensor("ident", [C, C], b16, kind="ExternalInput").ap()
    fold_d = nc.dram_tensor("foldmat", [2 * BLOC, BLOC], f32, kind="ExternalInput").ap()
    out_logZ = nc.dram_tensor("out_logZ", [1, BLOC], f32, kind="ExternalOutput").ap()
    out_esum = nc.dram_tensor("out_esum", [1, BLOC], f32, kind="ExternalOutput").ap()

    with ExitStack() as ctx:
        tc = ctx.enter_context(tile.TileContext(nc))
        const = ctx.enter_context(tc.tile_pool(name="const", bufs=1))
        qpool = ctx.enter_context(tc.tile_pool(name="q", bufs=1))
        chunks = ctx.enter_context(tc.tile_pool(name="chunks", bufs=3))
        natp = ctx.enter_context(tc.tile_pool(name="natp", bufs=2))
        state = ctx.enter_context(tc.tile_pool(name="state", bufs=3))
        misc = ctx.enter_context(tc.tile_pool(name="misc", bufs=2))
        ps_s = ctx.enter_context(tc.tile_pool(name="ps_s", bufs=2, space="PSUM"))
        ps_bc = ctx.enter_context(tc.tile_pool(name="ps_bc", bufs=2, space="PSUM"))
        ps_z = ctx.enter_context(tc.tile_pool(name="ps_z", bufs=1, space="PSUM"))

        # ---- constants ----
        trans_sb = const.tile([C, C], f32)
        nc.sync.dma_start(trans_sb[:], trans_d)
        transT_sb = const.tile([2 * C, C], f32)
        nc.sync.dma_start(transT_sb[C:2 * C, :], transT_d)
        W = const.tile([2 * C, 2 * C], b16)
        nc.vector.memset(W[:], 0.0)
        nc.scalar.activation(W[0:C, 0:C], trans_sb[:], AF.Exp)
        nc.scalar.activation(W[C:2 * C, C:2 * C], transT_sb[C:2 * C, :], AF.Exp)

        startend_sb = const.tile([2 * C, 1], f32)
        nc.sync.dma_start(startend_sb[:], startend_d)
        expSE = const.tile([2 * C, 1], f32)
        nc.scalar.activation(expSE[:], startend_sb[:], AF.Exp)

        ident_pair = const.tile([2 * C, C], b16)
        nc.sync.dma_start(ident_pair[C:2 * C, :], ident_d)
        fold_sb = const.tile([2 * BLOC, BLOC], f32)
        nc.sync.dma_start(fold_sb[:], fold_d)

        ones1 = const.tile([1, C], b16)
        nc.vector.memset(ones1[:], 1.0)
        ones64 = const.tile([C, 1], b16)
        nc.vector.memset(ones64[:], 1.0)
        neg_shift = const.tile([2 * C, 1], f32)
        nc.vector.memset(neg_shift[:], -SHIFT)
        scales = const.tile([1, RROWS * BLOC], b16)
        nc.vector.memset(scales[:], 1.0)

        # ---- Qpair: [128, H*BLOC] ----
        Qt = qpool.tile([2 * C, H * BLOC], b16)
        # stage boundaries: small first chunk so slot 1 starts early
        bounds = [0]
        pos = 0
        while pos < H:
            step = 8 if pos == 0 else min(TCH, H - pos)
            step = min(step, H - pos)
            pos += step
            bounds.append(pos)
        for ch in range(len(bounds) - 1):
            lo, hi = bounds[ch], bounds[ch + 1]
            et = chunks.tile([2 * C, (hi - lo) * BLOC], b16, tag="emis")
            nc.sync.dma_start(
                et[:].rearrange("p (t b) -> p t b", t=hi - lo),
                emisP[:, lo:hi, :],
            )
            nc.scalar.activation(
                Qt[:, lo * BLOC:hi * BLOC], et[:], AF.Exp,
                bias=neg_shift[:, :1],
            )

        def q_slice(k, c):
            lo = k * BLOC + c * BG
            return Qt[:, lo:lo + BG]

        # ---- numerator ----
        num_tch = min(NUM_TCH, H)
        num_dma_tch = min(NUM_DMA_TCH, H)
        n_numops = H // num_tch
        num_parts = const.tile([2 * BLOC, n_numops], f32)
        num_emitted = [0]
        _nat = {}

        def emit_num_op():
            i = num_emitted[0]
            if i >= n_numops:
                return
            num_emitted[0] += 1
            dch = (i * num_tch) // num_dma_tch
            if _nat.get("ch") != dch:
                en = natp.tile([2 * BLOC, num_dma_tch * C], b16, tag="en")
                nc.sync.dma_start(
                    en[:], emis_nat[:, dch * num_dma_tch * C:(dch + 1) * num_dma_tch * C])
                on = natp.tile([2 * BLOC, num_dma_tch * C], b16, tag="on")
                nc.sync.dma_start(
                    on[:], oh_nat[:, dch * num_dma_tch * C:(dch + 1) * num_dma_tch * C])
                _nat["ch"] = dch
                _nat["tiles"] = (en, on)
            en, on = _nat["tiles"]
            off = (i * num_tch - dch * num_dma_tch) * C
            scr = misc.tile([2 * BLOC, num_tch * C], b16, tag="numscr")
            nc.vector.tensor_tensor(scr[:], en[:, off:off + num_tch * C],
                                    on[:, off:off + num_tch * C], op=AluOpType.mult)
            scr2 = misc.tile([2 * BLOC, num_tch * C], b16, tag="numscr2")
            nc.scalar.activation(scr2[:], scr[:], AF.Copy,
                                 accum_out=num_parts[:, i:i + 1])

        # ---- init pair-chains (slot 0) ----
        p_cur = []
        for c in range(NCHAIN):
            p0 = state.tile([2 * C, BG], b16, tag=f"p{c}")
            nc.vector.tensor_scalar(p0[:], q_slice(0, c), expSE[:, :1], None,
                                    op0=AluOpType.mult)
            p_cur.append(p0)

        def renorm_prep(x_sb, row, c):
            """Extract power-of-2 scales from pair tile x rows 0 / C and
            broadcast them across partitions (runs off the critical path)."""
            srow_f = scales[:1, (2 * row) * BLOC + c * BG:(2 * row) * BLOC + c * BG + BG]
            srow_b = scales[:1, (2 * row + 1) * BLOC + c * BG:(2 * row + 1) * BLOC + c * BG + BG]
            nc.vector.tensor_scalar(srow_f.bitcast(u16), x_sb[:1, :].bitcast(u16),
                                    0x7F80, 0x7F80, op0=AluOpType.bitwise_and,
                                    op1=AluOpType.bitwise_xor)
            nc.vector.tensor_scalar(srow_b.bitcast(u16), x_sb[C:C + 1, :].bitcast(u16),
                                    0x7F80, 0x7F80, op0=AluOpType.bitwise_and,
                                    op1=AluOpType.bitwise_xor)
            bc = ps_bc.tile([2 * C, BG], f32, tag="bc")
            nc.tensor.matmul(bc[0:C, :], lhsT=ones1[:], rhs=srow_f,
                             start=True, stop=True)
            nc.tensor.matmul(bc[C:2 * C, :], lhsT=ones1[:], rhs=srow_b,
                             start=True, stop=True)
            return bc

        # ---- scan ----
        bc_pending = [None] * NCHAIN
        for k in range(1, H):
            for c in range(NCHAIN):
                s = ps_s.tile([2 * C, BG], f32, tag=f"s{c}")
                nc.tensor.matmul(s[:], lhsT=W[:], rhs=p_cur[c][:],
                                 start=True, stop=True)
                p_new = state.tile([2 * C, BG], b16, tag=f"p{c}")
                nc.vector.tensor_tensor(p_new[:], s[:], q_slice(k, c),
                                        op=AluOpType.mult)
                if k % K == 0:
                    bc = renorm_prep(p_new, k // K - 1, c)
                    p2 = state.tile([2 * C, BG], b16, tag=f"p{c}")
                    nc.vector.tensor_tensor(p2[:], p_new[:], bc[:],
                                            op=AluOpType.mult)
                    p_new = p2
                p_cur[c] = p_new
            if k % (H // n_numops) == (H // n_numops) - 1:
                emit_num_op()
        while num_emitted[0] < n_numops:
            emit_num_op()

        # ---- stitch: Z = sum_j alpha[j] * (E @ v)[j] per chain ----
        logZrow = misc.tile([1, BLOC], f32, tag="logZ")
        scales_ln = misc.tile([1, RROWS * BLOC], f32, tag="sln")
        nc.scalar.activation(scales_ln[:], scales[:1, :], AF.Ln)
        ssum = misc.tile([1, BLOC], f32, tag="ssum")
        nc.vector.tensor_reduce(
            ssum[:], scales_ln[:1, :].rearrange("p (r b) -> p b r", r=RROWS),
            mybir.AxisListType.X, AluOpType.add)
        for c in range(NCHAIN):
            s = ps_s.tile([2 * C, BG], f32, tag=f"s{c}")
            nc.tensor.matmul(s[:], lhsT=W[:], rhs=p_cur[c][:], start=True, stop=True)
            beta_hi = misc.tile([2 * C, BG], b16, tag="betahi")
            nc.vector.tensor_copy(beta_hi[C:2 * C, :], s[C:2 * C, :])
            blo = ps_bc.tile([C, BG], f32, tag="bc")
            nc.tensor.matmul(blo[:], lhsT=ident_pair[C:2 * C, :],
                             rhs=beta_hi[C:2 * C, :], start=True, stop=True)
            w = misc.tile([C, BG], b16, tag="w")
            nc.vector.tensor_tensor(w[:], blo[:], p_cur[c][0:C, :],
                                    op=AluOpType.mult)
            z = ps_z.tile([1, BG], f32, tag="z")
            nc.tensor.matmul(z[:], lhsT=ones64[:], rhs=w[:], start=True, stop=True)
            lnz = misc.tile([1, BG], f32, tag="lnz")
            nc.scalar.activation(lnz[:], z[:], AF.Ln)
            nc.vector.scalar_tensor_tensor(
                logZrow[:1, c * BG:(c + 1) * BG], lnz[:], float(SHIFT * T),
                ssum[:1, c * BG:(c + 1) * BG],
                op0=AluOpType.add, op1=AluOpType.subtract)
        nc.sync.dma_start(out_logZ, logZrow[:])

        # ---- numerator fold ----
        parts_red = misc.tile([2 * BLOC, 1], f32, tag="partsred")
        nc.vector.tensor_reduce(parts_red[:], num_parts[:], mybir.AxisListType.X,
                                AluOpType.add)
        ez = ps_z.tile([1, BLOC], f32, tag="z")
        nc.tensor.matmul(ez[:], lhsT=parts_red[:], rhs=fold_sb[:],
                         start=True, stop=True)
        esum_sb = misc.tile([1, BLOC], f32, tag="esum")
        nc.vector.tensor_copy(esum_sb[:], ez[:])
        nc.sync.dma_start(out_esum, esum_sb[:])

    nc.compile()
    return nc


_PROG_CACHE = {}


def _get_program(T_=T):
    if T_ not in _PROG_CACHE:
        _PROG_CACHE[T_] = build_crf_program(T=T_)
    return _PROG_CACHE[T_]


def host_prepare(emissions, tags, transitions, start_transitions, end_transitions,
                 T_=T):
    """Per-core input maps + host (tiny-tensor) numerator part."""
    H = T_ // 2
    in_maps = []
    trans_f = np.ascontiguousarray(transitions, dtype=np.float32)
    transT_f = np.ascontiguousarray(transitions.T, dtype=np.float32)
    startend = np.concatenate([start_transitions, end_transitions]).astype(
        np.float32).reshape(2 * C, 1)
    ident = np.eye(C, dtype=bf16)
    fold = np.tile(np.eye(BLOC, dtype=np.float32), (2, 1))
    cidx = np.arange(C, dtype=np.int32)
    tiny = np.zeros(B, np.float64)
    for c in range(NCORES):
        b0 = c * BLOC
        em = emissions[b0:b0 + BLOC, :T_, :]            # [Bl,T,C]
        emT = em.transpose(2, 1, 0)                     # [C,T,Bl]
        # top: t=0..H-1 ; bottom: t=T-1..H (time-reversed)
        emisP = np.concatenate([emT[:, :H, :], emT[:, ::-1, :][:, :H, :]], axis=0)
        emisP = np.ascontiguousarray(emisP).astype(bf16)
        emis_nat = np.ascontiguousarray(
            em.reshape(BLOC, 2, H * C).transpose(1, 0, 2).reshape(2 * BLOC, H * C)
        ).astype(bf16)
        tg = tags[b0:b0 + BLOC, :T_]                    # [Bl,T]
        oh = (tg[:, :, None] == cidx[None, None, :])    # [Bl,T,C]
        oh_nat = np.ascontiguousarray(
            oh.reshape(BLOC, 2, H * C).transpose(1, 0, 2).reshape(2 * BLOC, H * C)
        ).astype(bf16)
        in_maps.append({
            "emisP": emisP, "emis_nat": emis_nat, "oh_nat": oh_nat,
            "trans": trans_f, "transT": transT_f, "startend": startend,
            "ident": ident, "foldmat": fold,
        })
        tiny[b0:b0 + BLOC] = (
            start_transitions[tg[:, 0]].astype(np.float64)
            + np.take_along_axis(
                transitions[tg[:, :-1]], tg[:, 1:, None], axis=2)[:, :, 0].sum(1)
            + end_transitions[tg[:, -1]]
        )
    return in_maps, tiny


def kernel(emissions, tags, mask, transitions, start_transitions,
           end_transitions):
    from concourse.bass_utils import run_bass_kernel_spmd
    nc = _get_program()
    in_maps, tiny = host_prepare(emissions, tags, transitions,
                                 start_transitions, end_transitions)
    res = run_bass_kernel_spmd(nc, in_maps, core_ids=list(range(NCORES)))
    vals = np.zeros(B, np.float64)
    for c in range(NCORES):
        b0 = c * BLOC
        logZ = res.results[c]["out_logZ"].reshape(BLOC).astype(np.float64)
        esum = res.results[c]["out_esum"].reshape(BLOC).astype(np.float64)
        vals[b0:b0 + BLOC] = logZ - esum - tiny[b0:b0 + BLOC]
    return np.float32(np.mean(vals))
